# revision 26
# baseline (speedup 1.0000x reference)
"""GCN layer (fc + gather/scatter-sum) on 8 trn2 NeuronCores.

Math identity used: out = segment_sum(take(feature @ W + b, src), dst)
                        = segment_sum(take(feature, src), dst) @ W + deg * b
so the FC is folded AFTER edge aggregation. Each core owns a contiguous
range of dst nodes; its edges are host-sorted by (superbatch, src-window,
dst-block, src). Feature rows are gathered as bf16 (512B rows) from a
replicated DRAM table with the custom SWDGE dma_gather (1 descriptor per
edge). Desc-gen on the Q7 pairs (~8ns/desc, x4 queue pairs) is the
kernel's critical path, so padded descriptors are eliminated: per-core
slot layouts pack each (block, window) cell exactly (no tile rounding),
idx arrays end in -1 sentinels, and each call's num_idxs register is
reg_load-ed with that core's REAL edge count (the ucode truncates
trailing negative idxs before generating descriptors, and the decode
reserves ring space from the register, so padding costs ~0 Q7 time).
Because per-core cell offsets differ, each dst-block's matmul contracts
a STATIC tile range spanning all cores' offsets; the one-hot
A[e,d] = (dstrel_sb[e] == d + 128*j) is built from superbatch-absolute
fp16 dstrel (pads = -1), which zeroes foreign blocks' slots in shared
tiles. Registers rotate x8 so reg_load WARs never stall the in-order
Pool FIFO. The block FC runs on-chip: PE transpose, PSUM->SBUF copies
on the otherwise-idle ACT engine, two 128-contraction matmuls with W,
and bias = deg (x) b folded in as a rank-1 matmul into the same PSUM.
"""

import os
import numpy as np
import ml_dtypes

import concourse.bass as bass
import concourse.bacc as bacc
import concourse.mybir as mybir
from concourse import tile
from concourse import library_config

P = 128
NCORES = 8
NREGS = 8


def _patch_tile_exit():
    """The walrus build in this container rejects two constructs Tile emits
    at TileContext exit: a Drain carrying more than one sync wait ("Too many
    sync wait commands") and the sem_clear InstISA ("ISA wrong length").
    Replace the exit sequence with equivalent one-wait-per-Drain chains and
    skip the semaphore clears (fine for single-execution NEFFs)."""
    import bass_rust
    from concourse.vector_clock import ScopedClock

    def _drain_and_barrier(self, tick_clock, wait_clock):
        drain_inst = self.nc.sync.drain()
        wait_clock.add_sem_waits(
            drain_inst.ins, ScopedClock({None: tick_clock.global_clock})
        )
        si = drain_inst.ins.sync_info
        if si is not None and len(si.on_wait) > 1:
            waits = list(si.on_wait)
            drain_inst.ins.sync_info = bass_rust.SyncInfo(
                on_wait=waits[:1], on_update=list(si.on_update))
            for w in waits[1:]:
                extra = self.nc.sync.drain()
                extra.ins.sync_info = bass_rust.SyncInfo(
                    on_wait=[w], on_update=[])
        self.nc.all_engine_barrier()
        popped = self.nc._tile_sem_poison_stack.pop()
        assert popped is self._sem_poison
        self.nc.all_engine_barrier()

    tile.TileContext._drain_and_barrier = _drain_and_barrier


_patch_tile_exit()


class Cfg:
    def __init__(self, n_nodes, d_in, d_out, ncores, wbounds, sb_blocks):
        self.N = n_nodes
        self.D = d_in
        self.DO = d_out
        self.ncores = ncores
        self.wbounds = wbounds
        self.ngroups = len(wbounds) - 1
        assert max(b - a for a, b in zip(wbounds, wbounds[1:])) <= 32768
        self.npc = n_nodes // ncores       # nodes per core
        self.nblk = (self.npc + P - 1) // P
        self.sb = sb_blocks                # blocks per superbatch


FULL_CFG = Cfg(n_nodes=100000, d_in=256, d_out=64, ncores=8,
               wbounds=(0, 25000, 50000, 75000, 100000), sb_blocks=4)


def _sb_ranges(cfg):
    # last few superbatches shrink to 2 blocks so the end-of-kernel
    # pipeline drain (compute lags desc-gen by ~2 superbatches) is short
    sbr = []
    k0 = 0
    while k0 < cfg.nblk:
        rem = cfg.nblk - k0
        step = cfg.sb if rem > 6 else (2 if rem > 4 else 1)
        sbr.append(range(k0, min(k0 + step, cfg.nblk)))
        k0 += step
    return sbr


def _balance_blocks(src, dst, cfg, sbr):
    """Assign 128-dst blocks to (core, slot) so per-(superbatch, group)
    edge counts are near-equal across cores. Returns newpos[N]: old dst
    id -> new global row (core = newpos // npc)."""
    N, npc, ng = cfg.N, cfg.npc, cfg.ngroups
    nfull = (npc // P) * cfg.ncores
    small_sz = npc - (npc // P) * P
    fullN = nfull * P
    ids = np.arange(N)
    bid = np.where(ids < fullN, ids // P, nfull + (ids - fullN) // small_sz)
    grp = np.searchsorted(np.asarray(cfg.wbounds), src, side="right") - 1
    C = np.zeros((nfull + cfg.ncores, ng), dtype=np.int64)
    np.add.at(C, (bid[dst], grp), 1)

    rng = np.random.default_rng(12345)
    full_ids = rng.permutation(nfull)
    small_ids = list(range(nfull, nfull + cfg.ncores))
    slots = [[] for _ in range(cfg.ncores)]
    fp = 0
    for s, ks in enumerate(sbr):
        sz = len(ks)
        last = s == len(sbr) - 1
        take = 8 * sz - (8 if last else 0)
        pool = list(full_ids[fp:fp + take])
        fp += take
        if last:
            pool += small_ids
        blocks = sorted(pool, key=lambda b: -int(C[b].sum()))
        tgt = C[pool].sum(axis=0) / cfg.ncores
        load = np.zeros((cfg.ncores, ng))
        cap = np.full(cfg.ncores, sz)
        nsm = np.ones(cfg.ncores) if last else None
        for b in blocks:
            issm = b >= nfull
            best, bestv = -1, None
            for m in range(cfg.ncores):
                if cap[m] <= 0:
                    continue
                if last and issm and nsm[m] <= 0:
                    continue
                if last and not issm and cap[m] - nsm[m] <= 0:
                    continue
                v = float(np.max(load[m] + C[b] - tgt))
                if bestv is None or v < bestv:
                    best, bestv = m, v
            load[best] += C[b]
            cap[best] -= 1
            if last and issm:
                nsm[best] -= 1
            slots[best].append(b)
        # swap-repair: reduce max over (core, g) of load excess
        if not last:
            mine = [slots[m][-sz:] for m in range(cfg.ncores)]
            for _ in range(400):
                dev = load - tgt
                mbad = int(np.argmax(dev.max(axis=1)))
                cur = float(dev.max())
                bestsw, bestv = None, cur
                for b1 in mine[mbad]:
                    for m2 in range(cfg.ncores):
                        if m2 == mbad:
                            continue
                        for b2 in mine[m2]:
                            l1 = load[mbad] - C[b1] + C[b2] - tgt
                            l2 = load[m2] - C[b2] + C[b1] - tgt
                            rest = max(float(dev[m].max())
                                       for m in range(cfg.ncores)
                                       if m not in (mbad, m2))
                            v = max(float(l1.max()), float(l2.max()), rest)
                            if v < bestv - 1e-9:
                                bestv, bestsw = v, (b1, m2, b2)
                if bestsw is None:
                    break
                b1, m2, b2 = bestsw
                mine[mbad].remove(b1)
                mine[m2].remove(b2)
                mine[mbad].append(b2)
                mine[m2].append(b1)
                load[mbad] += C[b2] - C[b1]
                load[m2] += C[b1] - C[b2]
            for m in range(cfg.ncores):
                slots[m][-sz:] = mine[m]
    for m in range(cfg.ncores):
        sm = [b for b in slots[m] if b >= nfull]
        slots[m] = [b for b in slots[m] if b < nfull] + sm

    newpos = np.empty(N, dtype=np.int64)
    for m in range(cfg.ncores):
        row = m * npc
        for b in slots[m]:
            if b < nfull:
                lo, n = b * P, P
            else:
                lo, n = fullN + (b - nfull) * small_sz, small_sz
            newpos[lo:lo + n] = row + np.arange(n)
            row += n
        assert row == (m + 1) * npc
    return newpos


def _prep_host(feature, W, b, src, dst, cfg):
    """Shard + sort edges, build per-core packed slot arrays and the shared
    static call/tile-range structure."""
    N, npc, nblk, ng = cfg.N, cfg.npc, cfg.nblk, cfg.ngroups
    src = np.asarray(src, dtype=np.int64)
    dst = np.asarray(dst, dtype=np.int64)
    sbr = _sb_ranges(cfg)
    nsb = len(sbr)
    sb_of_blk = np.zeros(nblk, dtype=np.int64)
    for s, ks in enumerate(sbr):
        sb_of_blk[list(ks)] = s

    newpos = _balance_blocks(src, dst, cfg, sbr)
    ndst = newpos[dst]

    per_core = []
    counts = np.zeros((cfg.ncores, nblk, ng), dtype=np.int64)
    for m in range(cfg.ncores):
        lo, hi = m * npc, (m + 1) * npc
        mask = (ndst >= lo) & (ndst < hi)
        es = src[mask]
        ed = ndst[mask] - lo
        blk = ed >> 7
        grp = np.searchsorted(np.asarray(cfg.wbounds), es, side="right") - 1
        order = np.lexsort((es, blk, grp, sb_of_blk[blk]))
        es, ed, blk, grp = es[order], ed[order], blk[order], grp[order]
        np.add.at(counts[m], (blk, grp), 1)
        per_core.append((es, ed, blk, grp))

    # per-core exact prefix offsets within each (s, g) call
    off = np.zeros((cfg.ncores, nblk, ng), dtype=np.int64)
    for s, ks in enumerate(sbr):
        kl = list(ks)
        c = counts[:, kl, :]                      # [M, nk, ng]
        off[:, kl, :] = np.cumsum(c, axis=1) - c
    rcall = np.zeros((cfg.ncores, nsb, ng), dtype=np.int64)
    for s, ks in enumerate(sbr):
        rcall[:, s, :] = counts[:, list(ks), :].sum(axis=1)
    rmin = rcall.min(axis=0)    # [nsb, ng] min real count over cores

    # static per-call tile counts and per-(block, g) spanning col ranges
    T = np.maximum(1, -(-rcall.max(axis=0) // P))   # [nsb, ng] tiles
    lo_t = np.zeros((nblk, ng), dtype=np.int64)
    hi_t = np.zeros((nblk, ng), dtype=np.int64)
    for k in range(nblk):
        s = sb_of_blk[k]
        for g in range(ng):
            if counts[:, k, g].max() == 0:
                lo_t[k, g] = hi_t[k, g] = 0
                continue
            lo_t[k, g] = off[:, k, g].min() // P
            hi_t[k, g] = min(
                int(T[s, g]),
                -(-(off[:, k, g] + counts[:, k, g]).max() // P))

    # call layout: slot-col positions, A-col positions
    pos = {}        # (s, g) -> first slot col
    acol0 = {}      # (k, g) -> first A col within its call's A tile
    acols = {}      # (s, g) -> total A cols
    p0 = 0
    for s, ks in enumerate(sbr):
        for g in range(ng):
            pos[(s, g)] = p0
            p0 += int(T[s, g])
            a = 0
            for k in ks:
                acol0[(k, g)] = a
                a += int(hi_t[k, g] - lo_t[k, g])
            acols[(s, g)] = a
    tot = p0 * P
    ncalls = nsb * ng

    in_maps = []
    ftab = np.ascontiguousarray(feature.astype(ml_dtypes.bfloat16))
    wmat = np.ascontiguousarray(W.astype(np.float32))
    brow = np.ascontiguousarray(b.astype(ml_dtypes.bfloat16)[None, :])
    # iota512[p, d] = d (fp16; block j uses cols 128j..128j+128)
    iota = np.ascontiguousarray(
        np.tile(np.arange(cfg.sb * P, dtype=np.float32)[None, :],
                (P, 1)).astype(np.float16))
    ident = np.eye(P, dtype=np.float32)

    for m in range(cfg.ncores):
        es, ed, blk, grp = per_core[m]
        idx_arr = np.full(tot, -1, dtype=np.int16)
        dst_arr = np.full(tot, -1.0, dtype=np.float32)
        cnt_arr = np.zeros(ncalls, dtype=np.int32)
        # edges are globally sorted by (sb, g, blk, src): bounds per cell
        key = (sb_of_blk[blk] * ng + grp) * nblk + blk
        # reconstruct per-call segments via counts/off
        ptr = 0
        for s, ks in enumerate(sbr):
            for g in range(ng):
                base = pos[(s, g)] * P
                for k in ks:
                    n = int(counts[m, k, g])
                    if n == 0:
                        continue
                    sl = slice(ptr, ptr + n)
                    o = base + int(off[m, k, g])
                    idx_arr[o:o + n] = (es[sl] - cfg.wbounds[g]).astype(
                        np.int16)
                    dst_arr[o:o + n] = (ed[sl] - ks.start * P).astype(
                        np.float32)
                    ptr += n
                cnt_arr[s * ng + g] = rcall[m, s, g]
        assert ptr == len(es)
        # ucode rx/tx cores read the idx wrap from different 16-partition
        # groups (queue-dependent): replicate across all 128 partitions
        idx16 = np.ascontiguousarray(
            np.tile(idx_arr.reshape(tot // 16, 16).T, (P // 16, 1)))
        dstrel = np.ascontiguousarray(
            dst_arr.reshape(tot // P, P).T.astype(np.float16))
        deg = np.zeros(nblk * P, dtype=np.float32)
        np.add.at(deg, ed, 1.0)
        degrow = np.ascontiguousarray(deg[None, :].astype(ml_dtypes.bfloat16))
        in_maps.append({
            "ftab": ftab, "idx16": idx16, "dstrel": dstrel,
            "wmat": wmat, "brow": brow, "iota": iota, "ident": ident,
            "degrow": degrow,
            "cnt": np.ascontiguousarray(cnt_arr[None, :]),
        })

    meta = dict(T=T, lo_t=lo_t, hi_t=hi_t, pos=pos, acol0=acol0,
                acols=acols, tot=tot, nsb=nsb, sbr=sbr, ncalls=ncalls,
                rmin=rmin, newpos=newpos)
    return in_maps, meta


def _build_program(cfg, meta):
    N, D, DO, nblk, ng = cfg.N, cfg.D, cfg.DO, cfg.nblk, cfg.ngroups
    T, lo_t, hi_t = meta["T"], meta["lo_t"], meta["hi_t"]
    pos, acol0, acols = meta["pos"], meta["acol0"], meta["acols"]
    tot, nsb, sbr, ncalls = meta["tot"], meta["nsb"], meta["sbr"], meta["ncalls"]
    rmin = meta["rmin"]
    bf16, f32, i16 = mybir.dt.bfloat16, mybir.dt.float32, mybir.dt.int16
    fp16, i32 = mybir.dt.float16, mybir.dt.int32

    nc = bacc.Bacc(None, target_bir_lowering=False, num_swdge_queues=4)
    ftab = nc.dram_tensor("ftab", [N, D], bf16, kind="ExternalInput")
    idx16 = nc.dram_tensor("idx16", [P, tot // 16], i16, kind="ExternalInput")
    dstrel = nc.dram_tensor("dstrel", [P, tot // P], fp16, kind="ExternalInput")
    wmat = nc.dram_tensor("wmat", [D, DO], f32, kind="ExternalInput")
    brow = nc.dram_tensor("brow", [1, DO], bf16, kind="ExternalInput")
    iota = nc.dram_tensor("iota", [P, cfg.sb * P], fp16, kind="ExternalInput")
    ident = nc.dram_tensor("ident", [P, P], f32, kind="ExternalInput")
    degrow = nc.dram_tensor("degrow", [1, nblk * P], bf16, kind="ExternalInput")
    cnt = nc.dram_tensor("cnt", [1, ncalls], i32, kind="ExternalInput")
    out = nc.dram_tensor("out", [cfg.npc, DO], f32, kind="ExternalOutput")

    kchunks = D // P

    with tile.TileContext(nc) as tc:
        with (
            tc.tile_pool(name="const", bufs=1) as cpool,
            tc.tile_pool(name="gathf", bufs=3) as gpoolf,
            tc.tile_pool(name="degp", bufs=2) as degp,
            tc.tile_pool(name="amat", bufs=2) as apool,
            tc.tile_pool(name="work", bufs=4) as wpool,
            tc.tile_pool(name="psag", bufs=cfg.sb, space="PSUM") as psag,
            tc.tile_pool(name="pstr", bufs=2, space="PSUM") as pstr,
            tc.tile_pool(name="psout", bufs=2, space="PSUM") as psout,
        ):
            # rotating num_idxs registers, reg_load-ed with each core's real
            # edge counts. One BATCHED 4-reg load per superbatch (issued a
            # superbatch ahead) keeps the in-order Pool queue free of
            # per-call load stalls; x8 rotation = 2 superbatches of WAR
            # distance vs the gathers' decode-reads.
            cregs = [nc.alloc_register(mybir.EngineType.Pool, f"gc{i}")
                     for i in range(NREGS)]
            warm_reg = nc.alloc_register(mybir.EngineType.Pool, "gszwarm")
            nc.gpsimd.reg_mov(warm_reg, 16)

            def load_sb_regs(s):
                if s >= nsb:
                    return
                base = (s % 2) * 4
                nc.gpsimd.reg_load(cregs[base:base + ng],
                                   cntt[0:1, s * ng:(s + 1) * ng])

            # upload order: cnt + sb0's idx slice first (they gate the first
            # gathers), then a chunk covering the next few superbatches,
            # then the small consts, then dst table and the rest of the idxs
            cntt = cpool.tile([1, ncalls], i32)
            nc.sync.dma_start(out=cntt[:], in_=cnt[:])
            sb0_cols = sum(int(T[0, g]) for g in range(ng))
            sb0_end = sb0_cols * P
            idxt0 = cpool.tile([P, sb0_end // 16], i16)
            nc.sync.dma_start(out=idxt0[:], in_=idx16[:, 0:sb0_end // 16])
            idxt = cpool.tile([P, tot // 16], i16)
            s6_cols = sum(int(T[s, g]) for s in range(min(6, nsb))
                          for g in range(ng))
            mid = (s6_cols * P) // 16
            nc.sync.dma_start(out=idxt[:, sb0_end // 16:mid],
                              in_=idx16[:, sb0_end // 16:mid])
            iotat = cpool.tile([P, cfg.sb * P], fp16)
            nc.sync.dma_start(out=iotat[:], in_=iota[:])
            identt = cpool.tile([P, P], f32)
            nc.sync.dma_start(out=identt[:], in_=ident[:])
            bt = cpool.tile([1, DO], bf16)
            nc.sync.dma_start(out=bt[:], in_=brow[:])
            wts = []
            for c in range(kchunks):
                wt = cpool.tile([P, DO], f32, tag=f"w{c}")
                nc.sync.dma_start(out=wt[:], in_=wmat[c * P:(c + 1) * P, :])
                wts.append(wt)
            dstt = cpool.tile([P, tot // P], fp16)
            nc.sync.dma_start(out=dstt[:], in_=dstrel[:])
            nc.sync.dma_start(out=idxt[:, mid:], in_=idx16[:, mid:])

            def idx_slice(s, col0, ntiles):
                a, b = col0 * 8, (col0 + ntiles) * 8
                if s == 0:
                    return idxt0[:, a:b]
                return idxt[:, a:b]

            # tiny dummy gather at t~0: triggers the ~6us Q7 IRAM library
            # load so it overlaps the constant DMAs
            warm = wpool.tile([P, D], bf16, tag="warm")
            nc.gpsimd.dma_gather(
                out_ap=warm[:].rearrange("p (t e) -> p t e", e=D),
                in_ap=ftab[0:cfg.wbounds[1], :],
                idxs_ap=idxt0[:, 0:1],
                num_idxs=16, num_idxs_reg=warm_reg, elem_size=D,
                single_packet=True, queue_num=0)
            load_sb_regs(0)
            load_sb_regs(1)

            for s, ks in enumerate(sbr):
                gts = {}
                for g in range(ng):
                    tkc = int(T[s, g])
                    col0 = pos[(s, g)]
                    gt = gpoolf.tile([P, tkc * D], bf16, tag=f"g{g}")
                    gt3 = gt[:].rearrange("p (t e) -> p t e", e=D)
                    glo = cfg.wbounds[g]
                    ghi = min(cfg.wbounds[g + 1], N)
                    # slots beyond this core's real count are never written
                    # by the gather (descs truncated); clear the static tail
                    # region so A's zero rows don't meet NaN bit patterns
                    mn = int(rmin[s, g]) // P
                    if mn < tkc:
                        nc.vector.memset(gt[:, mn * D:], 0.0)
                    if os.environ.get("GCN_SKIP_GATHER"):
                        nc.vector.memset(gt[:, 0:1], 0.0)
                    else:
                        creg = cregs[(s % 2) * 4 + g]
                        q = (g + s) % 4
                        nc.gpsimd.dma_gather(
                            out_ap=gt3,
                            in_ap=ftab[glo:ghi, :],
                            idxs_ap=idx_slice(s, col0, tkc),
                            num_idxs=tkc * P,
                            num_idxs_reg=creg,
                            elem_size=D,
                            single_packet=(tkc * P <= 1024),
                            queue_num=q,
                        )
                    gts[g] = gt3
                load_sb_regs(s + 2)

                # one-hot tiles A[e, d] = (dstrel_sb[e] == d + 128*j), one
                # broadcast tensor_tensor per (block, group) over that
                # block's static spanning tile range. Foreign blocks' slots
                # and -1 pads compare unequal -> zero rows.
                abs_ = {}
                for g in range(ng):
                    nac = acols[(s, g)]
                    if nac == 0:
                        continue
                    ab = apool.tile([P, nac * P], bf16, tag=f"ab{g}")
                    col0 = pos[(s, g)]
                    for k in ks:
                        j = k - ks.start
                        lt, ht = int(lo_t[k, g]), int(hi_t[k, g])
                        nk = ht - lt
                        if nk == 0:
                            continue
                        a0 = acol0[(k, g)]
                        d_b = dstt[:, col0 + lt:col0 + ht].to_broadcast(
                            [P, nk, P])
                        iap = iotat[:, j * P:(j + 1) * P]
                        i_b = bass.AP(iap.tensor, iap.offset,
                                      [iap.ap[0], [0, nk], iap.ap[1]])
                        nc.vector.tensor_tensor(
                            out=ab[:, a0 * P:(a0 + nk) * P].rearrange(
                                "p (t d) -> p t d", d=P),
                            in0=i_b, in1=d_b, op=mybir.AluOpType.is_equal)
                    abs_[g] = ab

                # per-sb degree slice
                nks = len(ks)
                degsb = degp.tile([1, nks * P], bf16, tag="deg")
                nc.sync.dma_start(out=degsb[:],
                                  in_=degrow[0:1, ks.start * P:
                                             (ks.start + nks) * P])

                for k in ks:
                    j = k - ks.start
                    ps = psag.tile([P, D], f32, tag="agg")
                    agg_ap = ps[:, 0:D]
                    ntiles = sum(int(hi_t[k, g] - lo_t[k, g])
                                 for g in range(ng))
                    ti = 0
                    for g in range(ng):
                        lt, ht = int(lo_t[k, g]), int(hi_t[k, g])
                        if ht == lt:
                            continue
                        gt3 = gts[g]
                        ab = abs_[g]
                        a0 = acol0[(k, g)]
                        for t in range(lt, ht):
                            la = a0 + (t - lt)
                            amat = ab[:, la * P:(la + 1) * P]
                            first, last = ti == 0, ti == ntiles - 1
                            nc.tensor.matmul(agg_ap, lhsT=amat,
                                             rhs=gt3[:, t, :],
                                             start=first, stop=last)
                            ti += 1

                    # FC for this block: out_blk = agg @ W + deg (x) b
                    aggs = wpool.tile([P, D], f32, tag="aggs")
                    nc.scalar.copy(out=aggs[:], in_=agg_ap)
                    po = psout.tile([P, DO], f32, tag="po")
                    for c in range(kchunks):
                        pt = pstr.tile([P, P], f32, tag="pt")
                        nc.tensor.transpose(pt[:], aggs[:, c * P:(c + 1) * P],
                                            identt[:])
                        aT = wpool.tile([P, P], f32, tag="aT")
                        nc.scalar.copy(out=aT[:], in_=pt[:])
                        nc.tensor.matmul(po[:], lhsT=aT[:], rhs=wts[c][:],
                                         start=(c == 0), stop=False)
                    nc.tensor.matmul(po[:], lhsT=degsb[0:1, j * P:(j + 1) * P],
                                     rhs=bt[0:1, :], start=False, stop=True)
                    rows = min(P, cfg.npc - k * P)
                    outt = wpool.tile([P, DO], f32, tag="outt")
                    nc.scalar.copy(out=outt[:], in_=po[:])
                    nc.sync.dma_start(out=out[k * P:k * P + rows, :],
                                      in_=outt[:rows, :])
    return nc


def _run_spmd(nc, in_maps, trace=False):
    from concourse.bass_utils import run_bass_kernel_spmd
    return run_bass_kernel_spmd(nc, in_maps, list(range(len(in_maps))),
                                trace=trace)


_PROGRAM_CACHE = {}


def gcn_kernel(feature, W, b, src, dst, cfg=FULL_CFG, trace=False):
    in_maps, meta = _prep_host(feature, W, b, src, dst, cfg)
    key = (cfg.N, meta["tot"],
           tuple(np.asarray(meta["T"]).ravel().tolist()),
           tuple(np.asarray(meta["lo_t"]).ravel().tolist()),
           tuple(np.asarray(meta["hi_t"]).ravel().tolist()))
    nc = _PROGRAM_CACHE.get(key)
    if nc is None:
        nc = _build_program(cfg, meta)
        nc.finalize()
        _PROGRAM_CACHE[key] = nc
    res = _run_spmd(nc, in_maps, trace=trace)
    outs = [res.results[m]["out"] for m in range(cfg.ncores)]
    full = np.concatenate(outs, axis=0).astype(np.float32)
    full = full[meta["newpos"]]
    return full, res


def kernel(**inputs):
    feature = np.asarray(inputs["feature"], dtype=np.float32)
    W = np.asarray(inputs["W"], dtype=np.float32)
    b = np.asarray(inputs["b"], dtype=np.float32)
    src = np.asarray(inputs["src"], dtype=np.int32)
    dst = np.asarray(inputs["dst"], dtype=np.int32)
    full, _ = gcn_kernel(feature, W, b, src, dst, FULL_CFG)
    return full
